# revision 1
# baseline (speedup 1.0000x reference)
"""HardNegativeMiningContrastiveLoss on 8 trn2 NeuronCores (Bass/Tile).

Strategy:
  - Host: sort rows of both feature matrices by match_id. Since rows and
    columns share the same match_ids, the match matrix becomes block
    diagonal: all matches for (sorted) row i lie within +-(m*-1) columns
    of i, where m* = max id multiplicity. Each core owns a 512-row block
    of anchors for BOTH directions (v2t / t2v). The rhs (all 4096
    columns, transposed for matmul) is rotated per-core so the match
    band of local row-tile r sits at columns [128r, 128r+W) -- a uniform
    offset, which keeps the program SPMD.
  - Device: sim row-block via PE matmul (fp32), exp row-sums on ACT with
    fused accumulation, semi-hard range sums via fused
    scalar_tensor_tensor on DVE/GPSIMD, and all match-masked terms
    (mean_pos, corrections, the -log(p) keep terms) computed only on the
    narrow diagonal band.
  - Host: valid-row mask, final scalar reduction.
"""

import numpy as np

import concourse.bass as bass
import concourse.bacc as bacc
import concourse.tile as tile
from concourse import mybir
from concourse.bass_utils import run_bass_kernel_spmd
from contextlib import ExitStack

N_CORES = 8
B = 4096
D = 512
BLK = B // N_CORES  # 512 anchors per core
TEMPERATURE = 0.07
SEMI_HARD_MARGIN = 0.2
EPS = 1e-12

F32 = mybir.dt.float32
AX = mybir.AxisListType.X
ALU = mybir.AluOpType
ACTF = mybir.ActivationFunctionType

_CACHE = {}


def _build(shift: int, w: int, repeat: int = 1, loads_in_loop: bool = True):
    """Build + compile the SPMD program. w = band width, shift = column
    rotation applied on host (band of row-tile r = cols [128r, 128r+w)).
    repeat>1 replays the full load+compute pipeline (measurement only)."""
    nc = bacc.Bacc("TRN2", target_bir_lowering=False, debug=False,
                   num_devices=N_CORES)

    # Inputs (per-core values differ; shapes identical -> SPMD).
    rhs_t = nc.dram_tensor("rhs_t", [D, B], F32, kind="ExternalInput")
    rhs_v = nc.dram_tensor("rhs_v", [D, B], F32, kind="ExternalInput")
    ids_bcd = nc.dram_tensor("ids_bcd", [128, BLK + w], F32,
                             kind="ExternalInput")
    ids_rows = nc.dram_tensor("ids_rows", [128, 4], F32, kind="ExternalInput")
    inv_cnt = nc.dram_tensor("inv_cnt", [128, 4], F32, kind="ExternalInput")
    ks_out = nc.dram_tensor("ks_out", [128, 8], F32, kind="ExternalOutput")

    invT = float(1.0 / TEMPERATURE)
    NKC = D // 128   # 4 contraction chunks
    NCT = B // 512   # 8 column tiles
    NRT = BLK // 128  # 4 row tiles

    with tile.TileContext(nc) as tc, ExitStack() as ctx:
        rhs_pool = ctx.enter_context(tc.tile_pool(name="rhs", bufs=8))
        e_pool = ctx.enter_context(tc.tile_pool(name="erow", bufs=2))
        psum = ctx.enter_context(
            tc.tile_pool(name="psum", bufs=8, space=bass.MemorySpace.PSUM))
        scratch = ctx.enter_context(tc.tile_pool(name="scr", bufs=2))
        band_pool = ctx.enter_context(tc.tile_pool(name="band", bufs=3))
        small = ctx.enter_context(tc.tile_pool(name="small", bufs=6))
        const_pool = ctx.enter_context(tc.tile_pool(name="const", bufs=1))

        # Column ids broadcast across partitions (host-replicated).
        ids_bc = const_pool.tile([128, BLK + w], F32, tag="idsbc")
        nc.sync.dma_start(ids_bc[:], ids_bcd[:])

        # Per-row-tile ids / inv_cnt as [128,1] columns.
        ids_r = const_pool.tile([128, NRT], F32, tag="idsr")
        nc.sync.dma_start(ids_r[:], ids_rows[:])
        icnt_r = const_pool.tile([128, NRT], F32, tag="icntr")
        nc.sync.dma_start(icnt_r[:], inv_cnt[:])

        ks_cols = const_pool.tile([128, 2 * NRT], F32, tag="kscols")

        def load_rhs():
            rt_tiles, rv_tiles = [], []
            for k in range(NKC):
                t = rhs_pool.tile([128, B], F32, tag="rhs")
                nc.sync.dma_start(t[:], rhs_t[bass.ts(k, 128), :])
                rt_tiles.append(t)
            for k in range(NKC):
                t = rhs_pool.tile([128, B], F32, tag="rhs")
                nc.sync.dma_start(t[:], rhs_v[bass.ts(k, 128), :])
                rv_tiles.append(t)
            return rt_tiles, rv_tiles

        if not loads_in_loop:
            rt_tiles, rv_tiles = load_rhs()
        for rep in range(repeat):
          if loads_in_loop:
              rt_tiles, rv_tiles = load_rhs()

          for d in range(2):
              rh = rt_tiles if d == 0 else rv_tiles
              lsrc = rv_tiles if d == 0 else rt_tiles
              lh = [t[:, shift:shift + BLK] for t in lsrc]

              for r in range(NRT):
                  erow = e_pool.tile([128, B], F32, tag="erow")
                  sband = band_pool.tile([128, w], F32, tag="sband")
                  sl_e = small.tile([128, NCT], F32, tag="sl_e")
                  sl_1 = small.tile([128, NCT], F32, tag="sl_1")
                  sl_2 = small.tile([128, NCT], F32, tag="sl_2")

                  # mean_pos mask for the diagonal band.
                  bnd = slice(128 * r, 128 * r + w)
                  m_band = band_pool.tile([128, w], F32, tag="m")
                  nc.vector.tensor_scalar(
                      m_band[:], ids_bc[:, bnd], ids_r[:, r:r + 1], None,
                      op0=ALU.is_equal)
                  mp = small.tile([128, 1], F32, tag="mp")
                  mp2 = small.tile([128, 1], F32, tag="mp2")
                  pos_s = small.tile([128, 1], F32, tag="poss")

                  bsplit = min(128 * r + w, 512) - 128 * r  # band cols in c=0
                  nband = 1 if bsplit == w else 2

                  def do_matmul(c):
                      p = psum.tile([128, 512], F32, tag="p")
                      for k in range(NKC):
                          nc.tensor.matmul(
                              p[:], lh[k][:, bass.ts(r, 128)],
                              rh[k][:, bass.ts(c, 512)],
                              start=(k == 0), stop=(k == NKC - 1))
                      return p

                  def consume(c, p):
                      csl = bass.ts(c, 512)
                      nc.scalar.activation(
                          erow[:, csl], p[:], ACTF.Exp, scale=invT,
                          accum_out=sl_e[:, c:c + 1])
                      s1 = scratch.tile([128, 512], F32, tag="s1")
                      nc.vector.scalar_tensor_tensor(
                          out=s1[:], in0=p[:], scalar=mp[:],
                          in1=erow[:, csl], op0=ALU.is_lt, op1=ALU.mult,
                          accum_out=sl_1[:, c:c + 1])
                      s2 = scratch.tile([128, 512], F32, tag="s2")
                      nc.vector.scalar_tensor_tensor(
                          out=s2[:], in0=p[:], scalar=mp2[:],
                          in1=erow[:, csl], op0=ALU.is_le, op1=ALU.mult,
                          accum_out=sl_2[:, c:c + 1])

                  # Band col-tiles first: matmul, copy band slice to SBUF,
                  # derive mean_pos, then consume.
                  pheld = []
                  for c in range(nband):
                      p = do_matmul(c)
                      if c == 0:
                          nc.vector.tensor_copy(
                              sband[:, 0:bsplit], p[:, 128 * r:128 * r + bsplit])
                      else:
                          nc.vector.tensor_copy(
                              sband[:, bsplit:w], p[:, 0:w - bsplit])
                      pheld.append(p)
                  bscr = band_pool.tile([128, w], F32, tag="bscr")
                  nc.vector.scalar_tensor_tensor(
                      out=bscr[:], in0=m_band[:], scalar=0.0, in1=sband[:],
                      op0=ALU.add, op1=ALU.mult, accum_out=pos_s[:])
                  nc.vector.tensor_scalar(
                      mp[:], pos_s[:], icnt_r[:, r:r + 1], None, op0=ALU.mult)
                  nc.vector.tensor_scalar(
                      mp2[:], mp[:], SEMI_HARD_MARGIN, None, op0=ALU.subtract)
                  for c in range(nband):
                      consume(c, pheld[c])
                  for c in range(nband, NCT):
                      consume(c, do_matmul(c))

                  # Band corrections (match positions must not count as negs).
                  me = band_pool.tile([128, w], F32, tag="me")
                  g_e = small.tile([128, 1], F32, tag="ge")
                  nc.vector.scalar_tensor_tensor(
                      out=me[:], in0=m_band[:], scalar=0.0, in1=erow[:, bnd],
                      op0=ALU.add, op1=ALU.mult, accum_out=g_e[:])
                  g_1 = small.tile([128, 1], F32, tag="g1")
                  bs1 = band_pool.tile([128, w], F32, tag="bs1")
                  nc.vector.scalar_tensor_tensor(
                      out=bs1[:], in0=sband[:], scalar=mp[:], in1=me[:],
                      op0=ALU.is_lt, op1=ALU.mult, accum_out=g_1[:])
                  g_2 = small.tile([128, 1], F32, tag="g2")
                  bs2 = band_pool.tile([128, w], F32, tag="bs2")
                  nc.vector.scalar_tensor_tensor(
                      out=bs2[:], in0=sband[:], scalar=mp2[:], in1=me[:],
                      op0=ALU.is_le, op1=ALU.mult, accum_out=g_2[:])

                  # neg = sum(sl_e) + sum(sl_1) - sum(sl_2) - g_e - g_1 + g_2
                  red_a = small.tile([128, 1], F32, tag="reda")
                  nc.vector.reduce_sum(out=red_a[:], in_=sl_e[:], axis=AX)
                  red_b = small.tile([128, 1], F32, tag="redb")
                  nc.vector.reduce_sum(out=red_b[:], in_=sl_1[:], axis=AX)
                  red_c = small.tile([128, 1], F32, tag="redc")
                  nc.vector.reduce_sum(out=red_c[:], in_=sl_2[:], axis=AX)
                  t1 = small.tile([128, 1], F32, tag="t1")
                  nc.vector.tensor_tensor(out=t1[:], in0=red_a[:], in1=red_b[:],
                                          op=ALU.add)
                  t2 = small.tile([128, 1], F32, tag="t2")
                  nc.vector.tensor_tensor(out=t2[:], in0=red_c[:], in1=g_e[:],
                                          op=ALU.add)
                  t3 = small.tile([128, 1], F32, tag="t3")
                  nc.vector.tensor_tensor(out=t3[:], in0=t1[:], in1=t2[:],
                                          op=ALU.subtract)
                  t4 = small.tile([128, 1], F32, tag="t4")
                  nc.vector.tensor_tensor(out=t4[:], in0=t3[:], in1=g_1[:],
                                          op=ALU.subtract)
                  neg = small.tile([128, 1], F32, tag="neg")
                  nc.vector.tensor_tensor(out=neg[:], in0=t4[:], in1=g_2[:],
                                          op=ALU.add)

                  # keep terms: sum_match ln(E + neg) - sim/T
                  ea = band_pool.tile([128, w], F32, tag="ea")
                  nc.vector.tensor_scalar(ea[:], erow[:, bnd], neg[:], None,
                                          op0=ALU.add)
                  lg = band_pool.tile([128, w], F32, tag="lg")
                  nc.scalar.activation(lg[:], ea[:], ACTF.Ln)
                  ks_raw = small.tile([128, 1], F32, tag="ksraw")
                  bs3 = band_pool.tile([128, w], F32, tag="bs3")
                  nc.vector.scalar_tensor_tensor(
                      out=bs3[:], in0=m_band[:], scalar=0.0, in1=lg[:],
                      op0=ALU.add, op1=ALU.mult, accum_out=ks_raw[:])
                  pterm = small.tile([128, 1], F32, tag="pterm")
                  nc.vector.tensor_scalar(pterm[:], pos_s[:], invT, None,
                                          op0=ALU.mult)
                  nc.vector.tensor_tensor(
                      out=ks_cols[:, d * NRT + r:d * NRT + r + 1],
                      in0=ks_raw[:], in1=pterm[:], op=ALU.subtract)

        nc.sync.dma_start(ks_out[:], ks_cols[:])

    nc.compile()
    return nc


def _prep(vision_features, text_features, match_ids):
    v = np.ascontiguousarray(np.asarray(vision_features, dtype=np.float32))
    t = np.ascontiguousarray(np.asarray(text_features, dtype=np.float32))
    ids = np.asarray(match_ids).astype(np.int64)

    vn = v / np.maximum(np.linalg.norm(v, axis=1, keepdims=True), EPS)
    tn = t / np.maximum(np.linalg.norm(t, axis=1, keepdims=True), EPS)

    order = np.argsort(ids, kind="stable")
    ids_s = ids[order]
    _, inv, counts = np.unique(ids_s, return_inverse=True, return_counts=True)
    cnt_row = counts[inv].astype(np.int64)  # pos_cnt per sorted row
    m_star = int(cnt_row.max())

    shift = 16
    while m_star > shift + 1:
        shift += 16
    w = 128 + 2 * shift

    vT = np.ascontiguousarray(vn[order].T)  # [D, B]
    tT = np.ascontiguousarray(tn[order].T)
    ids_f = ids_s.astype(np.float32)
    inv_cnt = (1.0 / cnt_row).astype(np.float32)

    in_maps = []
    for core in range(N_CORES):
        roll = shift - core * BLK
        ic = np.roll(ids_f, roll)
        in_maps.append({
            "rhs_t": np.roll(tT, roll, axis=1),
            "rhs_v": np.roll(vT, roll, axis=1),
            "ids_bcd": np.ascontiguousarray(
                np.broadcast_to(ic[:BLK + w], (128, BLK + w))),
            "ids_rows": np.ascontiguousarray(
                ids_f[core * BLK:(core + 1) * BLK].reshape(4, 128).T),
            "inv_cnt": np.ascontiguousarray(
                inv_cnt[core * BLK:(core + 1) * BLK].reshape(4, 128).T),
        })
    meta = {
        "cnt_row": cnt_row,
        "num_pos": int(cnt_row.sum()),
        "valid": (cnt_row > 0) & (cnt_row < B),
        "shift": shift,
        "w": w,
    }
    return in_maps, meta


def _finalize(results, meta):
    ks_v = np.concatenate(
        [r["ks_out"][:, 0:4].T.reshape(-1) for r in results])
    ks_t = np.concatenate(
        [r["ks_out"][:, 4:8].T.reshape(-1) for r in results])
    valid = meta["valid"]
    v2t = np.where(valid, ks_v, 0.0).sum(dtype=np.float64)
    t2v = np.where(valid, ks_t, 0.0).sum(dtype=np.float64)
    num_pos = meta["num_pos"]
    if num_pos > 0:
        loss = (v2t + t2v) / (2.0 * max(num_pos, 1.0))
    else:
        loss = 0.0
    return np.float32(loss)


def kernel(vision_features, text_features, match_ids, _trace=False):
    in_maps, meta = _prep(vision_features, text_features, match_ids)
    key = (meta["shift"], meta["w"])
    if key not in _CACHE:
        _CACHE[key] = _build(*key)
    nc = _CACHE[key]
    res = run_bass_kernel_spmd(nc, in_maps, list(range(N_CORES)),
                               trace=_trace)
    out = _finalize(res.results, meta)
    if _trace:
        return out, res
    return out



# revision 3
# speedup vs baseline: 2.9063x; 2.9063x over previous
"""HardNegativeMiningContrastiveLoss on 8 trn2 NeuronCores (Bass/Tile).

Strategy:
  - Host: l2-normalize, sort rows of both feature matrices by match_id
    (match matrix becomes block-diagonal within a +-shift band), cast to
    bf16. Each core owns a 512-row anchor block for BOTH directions
    (v2t / t2v); the rhs (all 4096 columns, transposed) is rotated
    per-core so the match band of local row-tile r sits at columns
    [128r, 128r+w) -- uniform offset, SPMD program.
  - Device (per core):
      PE    : sim row-block via bf16 matmuls (1 cycle/row vs fp32's 4),
              k-outer order per 2048-col half (4 LDWEIGHTS per half).
      ACT   : exp(sim/T) -> bf16 erow, 2048 cols per instruction
              (PSUM 4-bank reads), plus exp(mean_pos/T) and the Ln of
              the keep terms.
      DVE   : semi-hard window sums in EXP SPACE (exp is monotone, so
              s < mp  <=>  e^{s/T} < e^{mp/T}); with all-bf16 SBUF
              operands the scalar_tensor_tensor runs in 4x_2p mode.
              neg = sum E*1[es<emp] + sum E*1[es>emp2] over non-matched
              -- the full-row sums include matched cols, corrected by
              two small band STTs (exact cancellation: same quantized
              values, same comparisons).
      Pool  : all small band ops (me, corrections, mean_pos chain,
              keep-term assembly) -- otherwise-idle engine.
  - Host: valid-row mask, final scalar reduction.
"""

import numpy as np
import ml_dtypes

import concourse.bass as bass
import concourse.bacc as bacc
import concourse.tile as tile
from concourse import mybir
from concourse.bass_utils import run_bass_kernel_spmd
from contextlib import ExitStack

N_CORES = 8
B = 4096
D = 512
BLK = B // N_CORES  # 512 anchors per core
TEMPERATURE = 0.07
SEMI_HARD_MARGIN = 0.2
EPS = 1e-12

F32 = mybir.dt.float32
BF16 = mybir.dt.bfloat16
AX = mybir.AxisListType.X
ALU = mybir.AluOpType
ACTF = mybir.ActivationFunctionType

_CACHE = {}


def _build(shift: int, w: int, repeat: int = 1, loads_in_loop: bool = True):
    """Build + compile the SPMD program. w = band width, shift = column
    rotation applied on host (band of row-tile r = cols [128r, 128r+w)).
    repeat>1 replays the full load+compute pipeline (measurement only)."""
    nc = bacc.Bacc("TRN2", target_bir_lowering=False, debug=False,
                   num_devices=N_CORES)

    rhs_t = nc.dram_tensor("rhs_t", [D, B], BF16, kind="ExternalInput")
    rhs_v = nc.dram_tensor("rhs_v", [D, B], BF16, kind="ExternalInput")
    ids_bcd = nc.dram_tensor("ids_bcd", [128, BLK + w], F32,
                             kind="ExternalInput")
    ids_rows = nc.dram_tensor("ids_rows", [128, 4], F32, kind="ExternalInput")
    inv_cnt = nc.dram_tensor("inv_cnt", [128, 4], F32, kind="ExternalInput")
    ks_out = nc.dram_tensor("ks_out", [128, 8], F32, kind="ExternalOutput")

    invT = float(1.0 / TEMPERATURE)
    EM02 = float(np.exp(-SEMI_HARD_MARGIN / TEMPERATURE))
    NKC = D // 128   # 4 contraction chunks
    NRT = BLK // 128  # 4 row tiles
    HW_ = 2048        # columns per psum half

    with tile.TileContext(nc) as tc, ExitStack() as ctx:
        rhs_pool = ctx.enter_context(tc.tile_pool(name="rhs", bufs=16))
        e_pool = ctx.enter_context(tc.tile_pool(name="erow", bufs=2))
        psum = ctx.enter_context(
            tc.tile_pool(name="psum", bufs=2, space=bass.MemorySpace.PSUM))
        junk_pool = ctx.enter_context(tc.tile_pool(name="junk", bufs=1))
        band_pool = ctx.enter_context(tc.tile_pool(name="band", bufs=2))
        small = ctx.enter_context(tc.tile_pool(name="small", bufs=2))
        const_pool = ctx.enter_context(tc.tile_pool(name="const", bufs=1))

        # Column ids broadcast across partitions (host-replicated).
        ids_bc = const_pool.tile([128, BLK + w], F32, tag="idsbc")
        nc.sync.dma_start(ids_bc[:], ids_bcd[:])

        # Per-row-tile ids / inv_cnt as [128,1] columns.
        ids_r = const_pool.tile([128, NRT], F32, tag="idsr")
        nc.sync.dma_start(ids_r[:], ids_rows[:])
        icnt_r = const_pool.tile([128, NRT], F32, tag="icntr")
        nc.sync.dma_start(icnt_r[:], inv_cnt[:])

        ks_cols = const_pool.tile([128, 2 * NRT], F32, tag="kscols")
        junk = junk_pool.tile([128, HW_], BF16, tag="junk")

        def load_rhs():
            rt_tiles, rv_tiles = [], []
            for k in range(NKC):
                t = rhs_pool.tile([128, B], BF16, tag="rhs")
                nc.sync.dma_start(t[:], rhs_t[bass.ts(k, 128), :])
                rt_tiles.append(t)
            for k in range(NKC):
                t = rhs_pool.tile([128, B], BF16, tag="rhs")
                nc.sync.dma_start(t[:], rhs_v[bass.ts(k, 128), :])
                rv_tiles.append(t)
            return rt_tiles, rv_tiles

        if not loads_in_loop:
            rt_tiles, rv_tiles = load_rhs()
        for rep in range(repeat):
          if loads_in_loop:
              rt_tiles, rv_tiles = load_rhs()

          for d in range(2):
              rh = rt_tiles if d == 0 else rv_tiles
              lsrc = rv_tiles if d == 0 else rt_tiles
              lh = [t[:, shift:shift + BLK] for t in lsrc]

              for r in range(NRT):
                  erow = e_pool.tile([128, B], BF16, tag="erow")
                  bnd = slice(128 * r, 128 * r + w)
                  idsr = ids_r[:, r:r + 1]

                  # --- matmuls: two 2048-col halves, k-outer (weights
                  # loaded once per k per half) ---
                  halves = []
                  for h in range(2):
                      p = psum.tile([128, HW_], F32, tag="p")
                      for k in range(NKC):
                          for cc in range(4):
                              nc.tensor.matmul(
                                  p[:, 512 * cc:512 * (cc + 1)],
                                  lh[k][:, bass.ts(r, 128)],
                                  rh[k][:, bass.ts(4 * h + cc, 512)],
                                  start=(k == 0), stop=(k == NKC - 1))
                      halves.append(p)
                  p01, p23 = halves

                  # --- mean_pos chain from the psum band ---
                  scr = band_pool.tile([128, w], F32, tag="scr")
                  pos_s = small.tile([128, 1], F32, tag="poss")
                  nc.vector.scalar_tensor_tensor(
                      out=scr[:], in0=ids_bc[:, bnd], scalar=idsr,
                      in1=p01[:, bnd], op0=ALU.is_equal, op1=ALU.mult,
                      accum_out=pos_s[:])
                  mp = small.tile([128, 1], F32, tag="mp")
                  nc.vector.tensor_scalar(
                      mp[:], pos_s[:], icnt_r[:, r:r + 1], None, op0=ALU.mult)
                  emp = small.tile([128, 1], F32, tag="emp")
                  nc.scalar.activation(emp[:], mp[:], ACTF.Exp, scale=invT)
                  emp2 = small.tile([128, 1], F32, tag="emp2")
                  nc.vector.tensor_scalar(
                      emp2[:], emp[:], EM02, None, op0=ALU.mult)

                  # --- exp + window sums (exp space, bf16, DVE 4x) ---
                  sl6 = small.tile([128, 6], F32, tag="sl6")
                  me = band_pool.tile([128, w], F32, tag="me")
                  for h, p in enumerate(halves):
                      csl = slice(HW_ * h, HW_ * (h + 1))
                      nc.scalar.activation(
                          erow[:, csl], p[:], ACTF.Exp, scale=invT)
                      if h == 0:
                          # matched exps (band) for keep terms + corrections
                          nc.vector.scalar_tensor_tensor(
                              out=me[:], in0=ids_bc[:, bnd], scalar=idsr,
                              in1=erow[:, bnd], op0=ALU.is_equal,
                              op1=ALU.mult)
                      for wi, (sc, op) in enumerate(
                              [(emp, ALU.is_lt), (emp2, ALU.is_gt)]):
                          nc.vector.scalar_tensor_tensor(
                              out=junk[:], in0=erow[:, csl], scalar=sc[:],
                              in1=erow[:, csl], op0=op, op1=ALU.mult,
                              accum_out=sl6[:, 2 * h + wi:2 * h + wi + 1])

                  # --- matched-column corrections (exact: same quantized
                  # values and comparisons as the full-row pass) ---
                  for wi, (sc, op) in enumerate(
                          [(emp, ALU.is_lt), (emp2, ALU.is_gt)]):
                      nc.vector.scalar_tensor_tensor(
                          out=scr[:], in0=me[:], scalar=sc[:], in1=me[:],
                          op0=op, op1=ALU.mult,
                          accum_out=sl6[:, 4 + wi:5 + wi])

                  # neg = sum(windows) - sum(corrections)
                  red_w = small.tile([128, 1], F32, tag="redw")
                  nc.vector.reduce_sum(out=red_w[:], in_=sl6[:, 0:4], axis=AX)
                  red_c = small.tile([128, 1], F32, tag="redc")
                  nc.vector.reduce_sum(out=red_c[:], in_=sl6[:, 4:6], axis=AX)
                  neg = small.tile([128, 1], F32, tag="neg")
                  nc.vector.tensor_tensor(out=neg[:], in0=red_w[:],
                                          in1=red_c[:], op=ALU.subtract)

                  # keep terms: sum_match ln(E + neg) - sim/T
                  ea = band_pool.tile([128, w], F32, tag="ea")
                  nc.vector.tensor_scalar(ea[:], me[:], neg[:], None,
                                          op0=ALU.add)
                  lg = band_pool.tile([128, w], F32, tag="lg")
                  nc.scalar.activation(lg[:], ea[:], ACTF.Ln)
                  scr2 = band_pool.tile([128, w], F32, tag="scr2")
                  ks_raw = small.tile([128, 1], F32, tag="ksraw")
                  nc.vector.scalar_tensor_tensor(
                      out=scr2[:], in0=ids_bc[:, bnd], scalar=idsr,
                      in1=lg[:], op0=ALU.is_equal, op1=ALU.mult,
                      accum_out=ks_raw[:])
                  pterm = small.tile([128, 1], F32, tag="pterm")
                  nc.vector.tensor_scalar(pterm[:], pos_s[:], invT, None,
                                          op0=ALU.mult)
                  nc.vector.tensor_tensor(
                      out=ks_cols[:, d * NRT + r:d * NRT + r + 1],
                      in0=ks_raw[:], in1=pterm[:], op=ALU.subtract)

        nc.sync.dma_start(ks_out[:], ks_cols[:])

    nc.compile()
    return nc


def _prep(vision_features, text_features, match_ids):
    v = np.ascontiguousarray(np.asarray(vision_features, dtype=np.float32))
    t = np.ascontiguousarray(np.asarray(text_features, dtype=np.float32))
    ids = np.asarray(match_ids).astype(np.int64)

    vn = v / np.maximum(np.linalg.norm(v, axis=1, keepdims=True), EPS)
    tn = t / np.maximum(np.linalg.norm(t, axis=1, keepdims=True), EPS)

    order = np.argsort(ids, kind="stable")
    ids_s = ids[order]
    _, inv, counts = np.unique(ids_s, return_inverse=True, return_counts=True)
    cnt_row = counts[inv].astype(np.int64)  # pos_cnt per sorted row
    m_star = int(cnt_row.max())

    shift = 16
    while m_star > shift + 1:
        shift += 16
    w = 128 + 2 * shift

    vT = np.ascontiguousarray(vn[order].T.astype(ml_dtypes.bfloat16))
    tT = np.ascontiguousarray(tn[order].T.astype(ml_dtypes.bfloat16))
    ids_f = ids_s.astype(np.float32)
    inv_cnt = (1.0 / cnt_row).astype(np.float32)

    in_maps = []
    for core in range(N_CORES):
        roll = shift - core * BLK
        ic = np.roll(ids_f, roll)
        in_maps.append({
            "rhs_t": np.roll(tT, roll, axis=1),
            "rhs_v": np.roll(vT, roll, axis=1),
            "ids_bcd": np.ascontiguousarray(
                np.broadcast_to(ic[:BLK + w], (128, BLK + w))),
            "ids_rows": np.ascontiguousarray(
                ids_f[core * BLK:(core + 1) * BLK].reshape(4, 128).T),
            "inv_cnt": np.ascontiguousarray(
                inv_cnt[core * BLK:(core + 1) * BLK].reshape(4, 128).T),
        })
    meta = {
        "cnt_row": cnt_row,
        "num_pos": int(cnt_row.sum()),
        "valid": (cnt_row > 0) & (cnt_row < B),
        "shift": shift,
        "w": w,
    }
    return in_maps, meta


def _finalize(results, meta):
    ks_v = np.concatenate(
        [r["ks_out"][:, 0:4].T.reshape(-1) for r in results])
    ks_t = np.concatenate(
        [r["ks_out"][:, 4:8].T.reshape(-1) for r in results])
    valid = meta["valid"]
    v2t = np.where(valid, ks_v, 0.0).sum(dtype=np.float64)
    t2v = np.where(valid, ks_t, 0.0).sum(dtype=np.float64)
    num_pos = meta["num_pos"]
    if num_pos > 0:
        loss = (v2t + t2v) / (2.0 * max(num_pos, 1.0))
    else:
        loss = 0.0
    return np.float32(loss)


def kernel(vision_features, text_features, match_ids, _trace=False):
    in_maps, meta = _prep(vision_features, text_features, match_ids)
    key = (meta["shift"], meta["w"])
    if key not in _CACHE:
        _CACHE[key] = _build(*key)
    nc = _CACHE[key]
    res = run_bass_kernel_spmd(nc, in_maps, list(range(N_CORES)),
                               trace=_trace)
    out = _finalize(res.results, meta)
    if _trace:
        return out, res
    return out
